# revision 1
# baseline (speedup 1.0000x reference)
"""Trainium2 Bass kernel for nn_MLoss_68066641707785 (topk_masking loss).

Computes, for x, y of shape [128, 43264, 5] (fp32):
    m        = (y[:,:,0] > 0.5)
    face_num = sum(m)
    scale    = 1 + 1/face_num
    diff_box = scale * sum(m * (x[:,:,1:5]-y[:,:,1:5])^2) / (face_num*4)
    bce      = -(t*log(p) + (1-t)*log(1-p)),  p = x[:,:,0], t = y[:,:,0]
    diff_c   = scale * sum(m * bce) / face_num
    diff_bg  = 0.5 * mean(-log(1-p))
    out      = diff_box + diff_c + diff_bg          (scalar fp32)

Strategy: pure data-parallel over the batch axis (16 batches per core x 8
cores).  The host first de-interleaves each tensor into a contiguous
confidence plane [B,N] and box plane [B,N,4] so every on-device access is
unit-stride (a stride-5 access pattern runs at ~0.5 elem/cycle on DVE and
~0.25 on ACT).  Each core streams its ~27.7MB through SBUF in T tiles and
reduces on-chip to six [128, T] partial-sum strips:
    aS : sum(m*t)            bS : sum(m*(1-t))      (aS+bS = face count)
    s1 : sum(m*t*ln(p))      s2 : sum(m*(1-t)*ln(1-p))
    se : sum(m * sum_c (x_c-y_c)^2)                 (box SE, masked)
    bg : sum(ln(1-p))                               (all cells)
Work is split across engines: ACT does ln/ln/square, DVE does the fused
compare-multiply-accumulate ops (scalar_tensor_tensor) and the channel
reduce, GpSimd takes the box subtract for some tiles to keep DVE below the
~85us DMA floor.  The host sums the 8 cores' strips in float64 and applies
the final scalar formula.
"""

import numpy as np

try:
    from concourse import bacc, bass, mybir, tile
    from concourse.bass_utils import run_bass_kernel_spmd
except ImportError:  # repo not on sys.path in a fresh grading dir
    import sys

    for _p in ("/opt/trn_rl_repo", "/root/.axon_site/_ro/trn_rl_repo"):
        if _p not in sys.path:
            sys.path.insert(0, _p)
    from concourse import bacc, bass, mybir, tile
    from concourse.bass_utils import run_bass_kernel_spmd

THRESH = 0.5
ALPHA = 0.5

B, N, C = 128, 43264, 5
M = 8                      # cores
BS = B // M                # 16 batches per core
P = 128                    # SBUF partitions
CELLS = BS * N // P        # 5408 cells per partition per core
T = 8                      # tiles per core
FT = CELLS // T            # 676 cells per partition per tile
NSTRIP = 5
GP_SUB_TILES = 8           # tiles whose box-subtract runs on GpSimd

_CACHE = {}


def _build():
    f32 = mybir.dt.float32
    AF = mybir.ActivationFunctionType
    OP = mybir.AluOpType
    AX = mybir.AxisListType

    nc = bacc.Bacc("TRN2", target_bir_lowering=False, debug=False, num_devices=M)
    xc_d = nc.declare_dram_parameter("xc", [P, CELLS], f32, isOutput=False)
    yc_d = nc.declare_dram_parameter("yc", [P, CELLS], f32, isOutput=False)
    xb_d = nc.declare_dram_parameter("xb", [P, 4 * CELLS], f32, isOutput=False)
    yb_d = nc.declare_dram_parameter("yb", [P, 4 * CELLS], f32, isOutput=False)
    o_d = nc.declare_dram_parameter("o", [NSTRIP, P, T], f32, isOutput=True)
    xc_ap, yc_ap, xb_ap, yb_ap, o_ap = xc_d[:], yc_d[:], xb_d[:], yb_d[:], o_d[:]

    with tile.TileContext(nc) as tc:
        with tc.tile_pool(name="io", bufs=3) as io, \
             tc.tile_pool(name="mid", bufs=2) as mid, \
             tc.tile_pool(name="acc", bufs=1) as accp:
            faceS = accp.tile([P, T], f32)
            s1S = accp.tile([P, T], f32)
            s2S = accp.tile([P, T], f32)
            seS = accp.tile([P, T], f32)
            bgS = accp.tile([P, T], f32)

            for j in range(T):
                p_t = io.tile([P, FT], f32, tag="p")
                nc.sync.dma_start(out=p_t[:], in_=xc_ap[:, bass.ts(j, FT)])
                t_t = io.tile([P, FT], f32, tag="t")
                nc.sync.dma_start(out=t_t[:], in_=yc_ap[:, bass.ts(j, FT)])
                xb_t = io.tile([P, 4 * FT], f32, tag="xb")
                nc.sync.dma_start(out=xb_t[:], in_=xb_ap[:, bass.ts(j, 4 * FT)])
                yb_t = io.tile([P, 4 * FT], f32, tag="yb")
                nc.sync.dma_start(out=yb_t[:], in_=yb_ap[:, bass.ts(j, 4 * FT)])

                # ---- confidence channel (all unit-stride) ----
                lp = mid.tile([P, FT], f32, tag="lp")
                nc.scalar.activation(lp[:], p_t[:], AF.Ln)
                lq = mid.tile([P, FT], f32, tag="lq")
                nc.scalar.activation(lq[:], p_t[:], AF.Ln, bias=1.0, scale=-1.0,
                                     accum_out=bgS[:, j:j + 1])
                m = mid.tile([P, FT], f32, tag="m")
                nc.vector.tensor_scalar(m[:], t_t[:], THRESH, 0.0, OP.is_gt,
                                        OP.add, accum_out=faceS[:, j:j + 1])
                a = mid.tile([P, FT], f32, tag="a")
                nc.vector.tensor_mul(a[:], m[:], t_t[:])
                b = mid.tile([P, FT], f32, tag="b")
                nc.vector.tensor_sub(b[:], m[:], a[:])
                scr1 = mid.tile([P, FT], f32, tag="scr")
                nc.vector.scalar_tensor_tensor(
                    scr1[:], a[:], 1.0, lp[:], OP.mult, OP.mult,
                    accum_out=s1S[:, j:j + 1])
                scr2 = mid.tile([P, FT], f32, tag="scr")
                nc.vector.scalar_tensor_tensor(
                    scr2[:], b[:], 1.0, lq[:], OP.mult, OP.mult,
                    accum_out=s2S[:, j:j + 1])

                # ---- box channels ----
                d = mid.tile([P, 4 * FT], f32, tag="d", bufs=3)
                sub_eng = nc.gpsimd if j % 4 != 3 else nc.vector
                sub_eng.tensor_sub(d[:], xb_t[:], yb_t[:])
                sq = mid.tile([P, 4 * FT], f32, tag="sq", bufs=3)
                nc.scalar.activation(sq[:], d[:], AF.Square)
                sec = mid.tile([P, FT], f32, tag="sec")
                nc.vector.tensor_reduce(
                    sec[:], sq[:].rearrange("p (f c) -> p f c", c=4),
                    axis=AX.X, op=OP.add)
                scr3 = mid.tile([P, FT], f32, tag="scr")
                nc.vector.scalar_tensor_tensor(
                    scr3[:], m[:], 1.0, sec[:], OP.mult, OP.mult,
                    accum_out=seS[:, j:j + 1])

            for k, strip in enumerate((faceS, s1S, s2S, seS, bgS)):
                nc.sync.dma_start(out=o_ap[k], in_=strip[:])

    nc.compile()
    return nc


def _get_nc():
    if "nc" not in _CACHE:
        _CACHE["nc"] = _build()
    return _CACHE["nc"]


def _in_maps(x, y):
    x = np.asarray(x, dtype=np.float32)
    y = np.asarray(y, dtype=np.float32)
    xc = np.ascontiguousarray(x[:, :, 0])
    yc = np.ascontiguousarray(y[:, :, 0])
    xb = np.ascontiguousarray(x[:, :, 1:5])
    yb = np.ascontiguousarray(y[:, :, 1:5])
    maps = []
    for i in range(M):
        sl = slice(i * BS, (i + 1) * BS)
        maps.append({
            "xc": xc[sl].reshape(P, CELLS),
            "yc": yc[sl].reshape(P, CELLS),
            "xb": xb[sl].reshape(P, 4 * CELLS),
            "yb": yb[sl].reshape(P, 4 * CELLS),
        })
    return maps


def _combine(outs):
    """outs: list of M arrays [NSTRIP, P, T] -> scalar fp32 loss."""
    tot = np.zeros(NSTRIP, dtype=np.float64)
    for o in outs:
        tot += o.astype(np.float64).reshape(NSTRIP, -1).sum(axis=1)
    face, s1, s2, se, bg = tot
    scale = 1.0 + 1.0 / face
    diff_box = scale * se / (face * 4.0)
    diff_c = scale * (-(s1 + s2)) / face
    diff_bg = ALPHA * (-bg) / (B * N)
    return np.asarray(diff_box + diff_c + diff_bg, dtype=np.float32)


def kernel(x, y, **run_kwargs):
    nc = _get_nc()
    res = run_bass_kernel_spmd(nc, _in_maps(x, y), core_ids=list(range(M)),
                               **run_kwargs)
    out = _combine([res.results[i]["o"] for i in range(M)])
    if run_kwargs:
        return out, res
    return out



# revision 2
# speedup vs baseline: 1.4724x; 1.4724x over previous
"""Trainium2 Bass kernel for nn_MLoss_68066641707785 (topk_masking loss).

Computes, for x, y of shape [128, 43264, 5] (fp32):
    m        = (y[:,:,0] > 0.5)
    face_num = sum(m)
    scale    = 1 + 1/face_num
    diff_box = scale * sum(m * (x[:,:,1:5]-y[:,:,1:5])^2) / (face_num*4)
    bce      = -(t*log(p) + (1-t)*log(1-p)),  p = x[:,:,0], t = y[:,:,0]
    diff_c   = scale * sum(m * bce) / face_num
    diff_bg  = 0.5 * mean(-log(1-p))
    out      = diff_box + diff_c + diff_bg          (scalar fp32)

Strategy: pure data-parallel over the batch axis (16 batches per core x 8
cores).  The tolerance (2e-2) leaves orders of magnitude of slack, so the
host downcasts everything to bf16 before upload, halving HBM traffic (the
kernel is memory-bound): ~13.8 MB/core streams in ~41 us at ~340 GB/s.

On-chip the work is spread so no engine exceeds the DMA floor:
  DVE   (all plain bf16 ops at 2x/4x; accum variants are 1x so none used):
        m = (t > .5) [TS 4x], u = m*t, v = m-u, p1 = u*ln(p),
        p2 = v*ln(1-p), box sub for ch 0-1, and the four mask-mults
        dm_c = d_c*m.
  GpSimd: box sub for ch 2-3.
  ACT   : ln(p), ln(1-p) [+free accum -> bg strip], Square(dm) over all 4
        channels in one op [+free accum -> se strip].
  TensorE (otherwise idle): ones-vector matmuls accumulate column sums of
        m, p1, p2 into three PSUM rows across all tiles (face, s1, s2).
The host sums strips/rows in float64 and applies the final scalar formula.
"""

import numpy as np

try:
    import ml_dtypes
    from concourse import bacc, bass, mybir, tile
    from concourse.bass_utils import run_bass_kernel_spmd
except ImportError:  # repo not on sys.path in a fresh grading dir
    import sys

    for _p in ("/opt/trn_rl_repo", "/root/.axon_site/_ro/trn_rl_repo"):
        if _p not in sys.path:
            sys.path.insert(0, _p)
    import ml_dtypes
    from concourse import bacc, bass, mybir, tile
    from concourse.bass_utils import run_bass_kernel_spmd

THRESH = 0.5
ALPHA = 0.5

B, N, C = 128, 43264, 5
M = 8                      # cores
BS = B // M                # 16 batches per core
P = 128                    # SBUF partitions
CELLS = BS * N // P        # 5408 cells per partition per core
T = 4                      # tiles per core
FT = CELLS // T            # 1352 cells per partition per tile
QW = 512                   # psum row width (one bank)
_CHUNKS = []
_off = 0
while _off < FT:
    _CHUNKS.append((_off, min(QW, FT - _off)))
    _off += QW

_CACHE = {}


def _build():
    f32 = mybir.dt.float32
    bf16 = mybir.dt.bfloat16
    AF = mybir.ActivationFunctionType
    OP = mybir.AluOpType

    nc = bacc.Bacc("TRN2", target_bir_lowering=False, debug=False, num_devices=M)
    xc_d = nc.declare_dram_parameter("xc", [P, CELLS], bf16, isOutput=False)
    yc_d = nc.declare_dram_parameter("yc", [P, CELLS], bf16, isOutput=False)
    xb_d = nc.declare_dram_parameter("xb", [P, 4 * CELLS], bf16, isOutput=False)
    yb_d = nc.declare_dram_parameter("yb", [P, 4 * CELLS], bf16, isOutput=False)
    on_d = nc.declare_dram_parameter("ones", [P, 1], bf16, isOutput=False)
    o_d = nc.declare_dram_parameter("o", [P, 2 * T], f32, isOutput=True)
    q_d = nc.declare_dram_parameter("q", [1, 3 * QW], f32, isOutput=True)

    with tile.TileContext(nc) as tc:
        with tc.tile_pool(name="io", bufs=3) as io, \
             tc.tile_pool(name="mid", bufs=2) as mid, \
             tc.tile_pool(name="acc", bufs=1) as accp, \
             tc.tile_pool(name="ps", bufs=1, space="PSUM") as ps:
            strips = accp.tile([P, 2 * T], f32)      # bg cols 0..T-1, se T..2T-1
            onesv = accp.tile([P, 1], bf16)
            nc.sync.dma_start(out=onesv[:], in_=on_d[:])
            pq_face = ps.tile([1, QW], f32)
            pq_s1 = ps.tile([1, QW], f32)
            pq_s2 = ps.tile([1, QW], f32)

            nmm = T * len(_CHUNKS)
            imm = 0
            for j in range(T):
                p_t = io.tile([P, FT], bf16, tag="p")
                nc.sync.dma_start(out=p_t[:], in_=xc_d[:, bass.ts(j, FT)])
                t_t = io.tile([P, FT], bf16, tag="t")
                nc.sync.dma_start(out=t_t[:], in_=yc_d[:, bass.ts(j, FT)])
                xb_t = io.tile([P, 4 * FT], bf16, tag="xb")
                nc.sync.dma_start(out=xb_t[:], in_=xb_d[:, bass.ts(j, 4 * FT)])
                yb_t = io.tile([P, 4 * FT], bf16, tag="yb")
                nc.sync.dma_start(out=yb_t[:], in_=yb_d[:, bass.ts(j, 4 * FT)])

                # ---- confidence channel ----
                lp = mid.tile([P, FT], bf16, tag="lp")
                nc.scalar.activation(lp[:], p_t[:], AF.Ln)
                lq = mid.tile([P, FT], bf16, tag="lq")
                nc.scalar.activation(lq[:], p_t[:], AF.Ln, bias=1.0, scale=-1.0,
                                     accum_out=strips[:, j:j + 1])
                m = mid.tile([P, FT], bf16, tag="m")
                nc.vector.tensor_scalar(m[:], t_t[:], THRESH, None, OP.is_gt)
                u = mid.tile([P, FT], bf16, tag="u")
                nc.vector.tensor_tensor(u[:], m[:], t_t[:], OP.mult)
                v = mid.tile([P, FT], bf16, tag="v")
                nc.vector.tensor_tensor(v[:], m[:], u[:], OP.subtract)
                p1 = mid.tile([P, FT], bf16, tag="p1")
                nc.vector.tensor_tensor(p1[:], u[:], lp[:], OP.mult)
                p2 = mid.tile([P, FT], bf16, tag="p2")
                nc.vector.tensor_tensor(p2[:], v[:], lq[:], OP.mult)

                # ---- box channels (planar segments: ch c at [c*FT,(c+1)*FT)) ----
                dA = mid.tile([P, 2 * FT], bf16, tag="dA")
                nc.vector.tensor_tensor(dA[:], xb_t[:, :2 * FT],
                                        yb_t[:, :2 * FT], OP.subtract)
                dB = mid.tile([P, 2 * FT], bf16, tag="dB")
                nc.gpsimd.tensor_sub(dB[:], xb_t[:, 2 * FT:], yb_t[:, 2 * FT:])
                dm = mid.tile([P, 4 * FT], bf16, tag="dm")
                for c in range(2):
                    nc.vector.tensor_tensor(dm[:, c * FT:(c + 1) * FT],
                                            dA[:, c * FT:(c + 1) * FT], m[:],
                                            OP.mult)
                for c in range(2):
                    nc.vector.tensor_tensor(dm[:, (2 + c) * FT:(3 + c) * FT],
                                            dB[:, c * FT:(c + 1) * FT], m[:],
                                            OP.mult)
                nc.scalar.activation(dm[:], dm[:], AF.Square,
                                     accum_out=strips[:, T + j:T + j + 1])

                # ---- TensorE column-sum accumulation (face, s1, s2) ----
                for (off, w) in _CHUNKS:
                    first = imm == 0
                    last = imm == nmm - 1
                    nc.tensor.matmul(pq_face[:, :w], onesv[:],
                                     m[:, off:off + w], start=first, stop=last,
                                     skip_group_check=True)
                    nc.tensor.matmul(pq_s1[:, :w], onesv[:],
                                     p1[:, off:off + w], start=first, stop=last,
                                     skip_group_check=True)
                    nc.tensor.matmul(pq_s2[:, :w], onesv[:],
                                     p2[:, off:off + w], start=first, stop=last,
                                     skip_group_check=True)
                    imm += 1

            qs = accp.tile([1, 3 * QW], f32)
            nc.scalar.activation(qs[:, 0:QW], pq_face[:], AF.Copy)
            nc.scalar.activation(qs[:, QW:2 * QW], pq_s1[:], AF.Copy)
            nc.scalar.activation(qs[:, 2 * QW:3 * QW], pq_s2[:], AF.Copy)
            nc.sync.dma_start(out=o_d[:], in_=strips[:])
            nc.sync.dma_start(out=q_d[:], in_=qs[:])

    nc.compile()
    return nc


def _get_nc():
    if "nc" not in _CACHE:
        _CACHE["nc"] = _build()
    return _CACHE["nc"]


def _pack_core(x_sl, y_sl):
    """x_sl, y_sl: [BS, N, 5] fp32 -> bf16 planes for one core."""
    bf = ml_dtypes.bfloat16
    out = {}
    for name, a in (("x", x_sl), ("y", y_sl)):
        conf = np.ascontiguousarray(a[:, :, 0]).reshape(P, CELLS).astype(bf)
        box = a[:, :, 1:5].reshape(P, T, FT, 4)
        box = np.ascontiguousarray(box.transpose(0, 1, 3, 2))  # [P,T,4,FT]
        out[name + "c"] = conf
        out[name + "b"] = box.reshape(P, 4 * CELLS).astype(bf)
    return {"xc": out["xc"], "yc": out["yc"], "xb": out["xb"], "yb": out["yb"],
            "ones": np.ones((P, 1), bf)}


def _in_maps(x, y):
    x = np.asarray(x, dtype=np.float32)
    y = np.asarray(y, dtype=np.float32)
    maps = []
    for i in range(M):
        sl = slice(i * BS, (i + 1) * BS)
        maps.append(_pack_core(x[sl], y[sl]))
    return maps


def _combine(outs):
    """outs: list of M (o [P, 2T], q [1, 3*QW]) -> scalar fp32 loss."""
    bg = s1 = s2 = se = face = 0.0
    for o, q in outs:
        o = o.astype(np.float64)
        q = q.astype(np.float64)
        bg += o[:, :T].sum()
        se += o[:, T:].sum()
        face += q[0, 0:QW].sum()
        s1 += q[0, QW:2 * QW].sum()
        s2 += q[0, 2 * QW:3 * QW].sum()
    scale = 1.0 + 1.0 / face
    diff_box = scale * se / (face * 4.0)
    diff_c = scale * (-(s1 + s2)) / face
    diff_bg = ALPHA * (-bg) / (B * N)
    return np.asarray(diff_box + diff_c + diff_bg, dtype=np.float32)


def kernel(x, y, **run_kwargs):
    nc = _get_nc()
    res = run_bass_kernel_spmd(nc, _in_maps(x, y), core_ids=list(range(M)),
                               **run_kwargs)
    out = _combine([(res.results[i]["o"], res.results[i]["q"])
                    for i in range(M)])
    if run_kwargs:
        return out, res
    return out


# revision 4
# speedup vs baseline: 1.8429x; 1.2516x over previous
"""Trainium2 Bass kernel for nn_MLoss_68066641707785 (topk_masking loss).

Computes, for x, y of shape [128, 43264, 5] (fp32):
    m        = (y[:,:,0] > 0.5)
    face_num = sum(m)
    scale    = 1 + 1/face_num
    diff_box = scale * sum(m * (x[:,:,1:5]-y[:,:,1:5])^2) / (face_num*4)
    bce      = -(t*log(p) + (1-t)*log(1-p)),  p = x[:,:,0], t = y[:,:,0]
    diff_c   = scale * sum(m * bce) / face_num
    diff_bg  = 0.5 * mean(-log(1-p))
    out      = diff_box + diff_c + diff_bg          (scalar fp32)

Strategy: pure data-parallel over the batch axis (16 batches per core x 8
cores).  The tolerance (2e-2) leaves orders of magnitude of slack, so the
host downcasts everything to bf16 before upload, halving HBM traffic (the
kernel is memory-bound): ~13.8 MB/core streams in ~41 us at ~340 GB/s.

On-chip the work is spread so no engine exceeds the DMA floor:
  DVE   (all plain bf16 ops at 2x/4x; accum variants are 1x so none used):
        m = (t > .5) [TS 4x], u = m*t, v = m-u, p1 = u*ln(p),
        p2 = v*ln(1-p), box sub for ch 0-1, and the four mask-mults
        dm_c = d_c*m.
  GpSimd: box sub for ch 2-3.
  ACT   : ln(p), ln(1-p) [+free accum -> bg strip], Square(dm) over all 4
        channels in one op [+free accum -> se strip].
  TensorE (otherwise idle): ones-vector matmuls accumulate column sums of
        m, p1, p2 into three PSUM rows across all tiles (face, s1, s2).
The host sums strips/rows in float64 and applies the final scalar formula.
"""

import numpy as np

try:
    import ml_dtypes
    from concourse import bacc, bass, mybir, tile
    from concourse.bass_utils import run_bass_kernel_spmd
except ImportError:  # repo not on sys.path in a fresh grading dir
    import sys

    for _p in ("/opt/trn_rl_repo", "/root/.axon_site/_ro/trn_rl_repo"):
        if _p not in sys.path:
            sys.path.insert(0, _p)
    import ml_dtypes
    from concourse import bacc, bass, mybir, tile
    from concourse.bass_utils import run_bass_kernel_spmd

THRESH = 0.5
ALPHA = 0.5

B, N, C = 128, 43264, 5
M = 8                      # cores
BS = B // M                # 16 batches per core
P = 128                    # SBUF partitions
CELLS = BS * N // P        # 5408 cells per partition per core
T = 4                      # tiles per core
FT = CELLS // T            # 1352 cells per partition per tile
QW = 512                   # psum row width (one bank)
_CHUNKS = []
_off = 0
while _off < FT:
    _CHUNKS.append((_off, min(QW, FT - _off)))
    _off += QW

_CACHE = {}


def _build():
    f32 = mybir.dt.float32
    bf16 = mybir.dt.bfloat16
    AF = mybir.ActivationFunctionType
    OP = mybir.AluOpType

    nc = bacc.Bacc("TRN2", target_bir_lowering=False, debug=False, num_devices=M)
    xc_d = nc.declare_dram_parameter("xc", [P, CELLS], bf16, isOutput=False)
    yc_d = nc.declare_dram_parameter("yc", [P, CELLS], bf16, isOutput=False)
    xb_d = nc.declare_dram_parameter("xb", [P, 4 * CELLS], bf16, isOutput=False)
    yb_d = nc.declare_dram_parameter("yb", [P, 4 * CELLS], bf16, isOutput=False)
    on_d = nc.declare_dram_parameter("ones", [P, 1], bf16, isOutput=False)
    o_d = nc.declare_dram_parameter("o", [P, 2 * T], f32, isOutput=True)
    q_d = nc.declare_dram_parameter("q", [1, 3 * QW], f32, isOutput=True)

    with tile.TileContext(nc) as tc:
        with tc.tile_pool(name="io", bufs=3) as io, \
             tc.tile_pool(name="mid", bufs=2) as mid, \
             tc.tile_pool(name="acc", bufs=1) as accp, \
             tc.tile_pool(name="ps", bufs=1, space="PSUM") as ps:
            strips = accp.tile([P, 2 * T], f32)      # bg cols 0..T-1, se T..2T-1
            onesv = accp.tile([P, 1], bf16)
            nc.sync.dma_start(out=onesv[:], in_=on_d[:])
            pq_face = ps.tile([1, QW], f32)
            pq_s1 = ps.tile([1, QW], f32)
            pq_s2 = ps.tile([1, QW], f32)

            nmm = T * len(_CHUNKS)
            imm = 0
            for j in range(T):
                t_t = io.tile([P, FT], bf16, tag="t")
                nc.sync.dma_start(out=t_t[:], in_=yc_d[:, bass.ts(j, FT)])
                p_t = io.tile([P, FT], bf16, tag="p")
                nc.sync.dma_start(out=p_t[:], in_=xc_d[:, bass.ts(j, FT)])
                xb_t = io.tile([P, 4 * FT], bf16, tag="xb")
                nc.sync.dma_start(out=xb_t[:], in_=xb_d[:, bass.ts(j, 4 * FT)])
                yb_t = io.tile([P, 4 * FT], bf16, tag="yb")
                nc.sync.dma_start(out=yb_t[:], in_=yb_d[:, bass.ts(j, 4 * FT)])

                # ---- confidence channel ----
                lp = mid.tile([P, FT], bf16, tag="lp")
                nc.scalar.activation(lp[:], p_t[:], AF.Ln)
                lq = mid.tile([P, FT], bf16, tag="lq")
                nc.scalar.activation(lq[:], p_t[:], AF.Ln, bias=1.0, scale=-1.0,
                                     accum_out=strips[:, j:j + 1])
                m = mid.tile([P, FT], bf16, tag="m")
                nc.vector.tensor_scalar(m[:], t_t[:], THRESH, None, OP.is_gt)
                u = mid.tile([P, FT], bf16, tag="u")
                nc.vector.tensor_tensor(u[:], m[:], t_t[:], OP.mult)
                v = mid.tile([P, FT], bf16, tag="v")
                nc.vector.tensor_tensor(v[:], m[:], u[:], OP.subtract)
                p1 = mid.tile([P, FT], bf16, tag="p1")
                nc.vector.tensor_tensor(p1[:], u[:], lp[:], OP.mult)
                p2 = mid.tile([P, FT], bf16, tag="p2")
                nc.vector.tensor_tensor(p2[:], v[:], lq[:], OP.mult)

                # ---- box channels (planar segments: ch c at [c*FT,(c+1)*FT)) ----
                d4 = mid.tile([P, 4 * FT], bf16, tag="d4")
                nc.vector.tensor_tensor(d4[:], xb_t[:], yb_t[:], OP.subtract)
                dm = mid.tile([P, 4 * FT], bf16, tag="dm")
                for c in range(4):
                    nc.vector.tensor_tensor(dm[:, c * FT:(c + 1) * FT],
                                            d4[:, c * FT:(c + 1) * FT], m[:],
                                            OP.mult)
                nc.scalar.activation(dm[:], dm[:], AF.Square,
                                     accum_out=strips[:, T + j:T + j + 1])

                # ---- TensorE column-sum accumulation (face, s1, s2) ----
                for (off, w) in _CHUNKS:
                    first = imm == 0
                    last = imm == nmm - 1
                    nc.tensor.matmul(pq_face[:, :w], onesv[:],
                                     m[:, off:off + w], start=first, stop=last,
                                     skip_group_check=True)
                    nc.tensor.matmul(pq_s1[:, :w], onesv[:],
                                     p1[:, off:off + w], start=first, stop=last,
                                     skip_group_check=True)
                    nc.tensor.matmul(pq_s2[:, :w], onesv[:],
                                     p2[:, off:off + w], start=first, stop=last,
                                     skip_group_check=True)
                    imm += 1

            qs = accp.tile([1, 3 * QW], f32)
            nc.scalar.activation(qs[:, 0:QW], pq_face[:], AF.Copy)
            nc.scalar.activation(qs[:, QW:2 * QW], pq_s1[:], AF.Copy)
            nc.scalar.activation(qs[:, 2 * QW:3 * QW], pq_s2[:], AF.Copy)
            nc.sync.dma_start(out=o_d[:], in_=strips[:])
            nc.sync.dma_start(out=q_d[:], in_=qs[:])

    nc.compile()
    return nc


def _get_nc():
    if "nc" not in _CACHE:
        _CACHE["nc"] = _build()
    return _CACHE["nc"]


def _pack_core(x_sl, y_sl):
    """x_sl, y_sl: [BS, N, 5] fp32 -> bf16 planes for one core."""
    bf = ml_dtypes.bfloat16
    out = {}
    for name, a in (("x", x_sl), ("y", y_sl)):
        conf = np.ascontiguousarray(a[:, :, 0]).reshape(P, CELLS).astype(bf)
        box = a[:, :, 1:5].reshape(P, T, FT, 4)
        box = np.ascontiguousarray(box.transpose(0, 1, 3, 2))  # [P,T,4,FT]
        out[name + "c"] = conf
        out[name + "b"] = box.reshape(P, 4 * CELLS).astype(bf)
    return {"xc": out["xc"], "yc": out["yc"], "xb": out["xb"], "yb": out["yb"],
            "ones": np.ones((P, 1), bf)}


def _in_maps(x, y):
    x = np.asarray(x, dtype=np.float32)
    y = np.asarray(y, dtype=np.float32)
    maps = []
    for i in range(M):
        sl = slice(i * BS, (i + 1) * BS)
        maps.append(_pack_core(x[sl], y[sl]))
    return maps


def _combine(outs):
    """outs: list of M (o [P, 2T], q [1, 3*QW]) -> scalar fp32 loss."""
    bg = s1 = s2 = se = face = 0.0
    for o, q in outs:
        o = o.astype(np.float64)
        q = q.astype(np.float64)
        bg += o[:, :T].sum()
        se += o[:, T:].sum()
        face += q[0, 0:QW].sum()
        s1 += q[0, QW:2 * QW].sum()
        s2 += q[0, 2 * QW:3 * QW].sum()
    scale = 1.0 + 1.0 / face
    diff_box = scale * se / (face * 4.0)
    diff_c = scale * (-(s1 + s2)) / face
    diff_bg = ALPHA * (-bg) / (B * N)
    return np.asarray(diff_box + diff_c + diff_bg, dtype=np.float32)


def kernel(x, y, **run_kwargs):
    nc = _get_nc()
    res = run_bass_kernel_spmd(nc, _in_maps(x, y), core_ids=list(range(M)),
                               **run_kwargs)
    out = _combine([(res.results[i]["o"], res.results[i]["q"])
                    for i in range(M)])
    if run_kwargs:
        return out, res
    return out
